# revision 2
# baseline (speedup 1.0000x reference)
"""Multi-head attention (B=4, S=2048, D=1024, H=16, Dh=64) on 8 trn2 NeuronCores.

Sharding: core c -> (batch b = c//2, head-group g = c%2 of 8 heads).
Each core computes q/k/v projections for its 8 heads and the full attention,
writing o[b, :, 512*g : 512*(g+1)].  No collectives needed: the output's
feature dim is just the concatenation of per-head outputs.

Layout strategy (per core):
  - Host pre-transposes X (seq-major -> D-major) so the contraction dim D
    lands on SBUF partitions without on-chip transposes, and casts to bf16.
  - Projections compute qT/kT in [dh, seq] orientation (lhsT = W k-tile,
    rhs = X.T k-tile) and v in natural [seq, dh] orientation.
  - Scores are computed TRANSPOSED (scoresT[sk, sq] = kT.T @ qT) so that the
    exp'd attention matrix A.T is already in the [sk-partition, sq-free]
    layout the PV matmul needs as its stationary operand -> no transposes.
  - Softmax denominators come for free from a mask column appended to V
    (o_psum column 64 = sum over valid sk of A.T), so no reductions.
  - k-masking: V rows beyond V_len are zeroed on host and the mask column is
    0 there, so invalid sk contribute nothing to numerator or denominator.
    exp is computed without max-subtraction (scores are O(+-10), safe in
    fp32) which matches softmax exactly up to rounding.
  - q-masking + normalization fused: out_tile = o_psum[:, :64] *
    (maskq / sum) as a per-partition scalar multiply.

The program is compiled for SQT/SKT = ceil(max(Q_len)/128), ceil(max(V_len)/128)
tiles (shared SPMD program across the 8 cores), so work scales with the
actual sequence lengths.  Per-core smaller lengths are handled by the masks.
"""

import math

import numpy as np
import ml_dtypes


def _ensure_paths():
    import sys
    try:
        import concourse  # noqa: F401
        return
    except ImportError:
        pass
    for p in ("/opt/trn_rl_repo", "/root/.axon_site/_ro/trn_rl_repo"):
        if p not in sys.path:
            sys.path.insert(0, p)
    import concourse  # noqa: F401


P = 128          # SBUF partitions
D = 1024         # model dim
DH = 64          # head dim
HL = 8           # heads per core
E = HL * DH      # per-core output feature width (512)
NCORES = 8

_PROG_CACHE = {}

# exposed for test.py profiling reruns
_last_nc = None
_last_in_maps = None


def _build_program(SQT, SKT):
    """Build + bacc-compile the shared SPMD program for given tile counts."""
    _ensure_paths()
    import concourse.bass as bass  # noqa: F401
    import concourse.tile as tile
    from concourse import bacc, mybir

    BF = mybir.dt.bfloat16
    F32 = mybir.dt.float32
    Exp = mybir.ActivationFunctionType.Exp

    SQ = SQT * P
    SK = SKT * P
    QC = math.ceil(SQ / 512)   # sq chunks for matmul free dim / psum banks
    KC = math.ceil(SK / 512)
    KT = D // P                # 8 contraction tiles

    nc = bacc.Bacc("TRN2", target_bir_lowering=False, debug=False,
                   num_devices=NCORES)

    xqt = nc.dram_tensor("xqt", [D, SQ], BF, kind="ExternalInput").ap()
    xkt = nc.dram_tensor("xkt", [D, SK], BF, kind="ExternalInput").ap()
    xvt = nc.dram_tensor("xvt", [D, SK], BF, kind="ExternalInput").ap()
    wq = nc.dram_tensor("wq", [D, E], BF, kind="ExternalInput").ap()
    wk = nc.dram_tensor("wk", [D, E], BF, kind="ExternalInput").ap()
    wv = nc.dram_tensor("wv", [D, E], BF, kind="ExternalInput").ap()
    maskq = nc.dram_tensor("maskq", [SQT, P], F32, kind="ExternalInput").ap()
    maskk8 = nc.dram_tensor("maskk8", [SK, HL, 1], BF, kind="ExternalInput").ap()
    out = nc.dram_tensor("out", [2048, E], F32, kind="ExternalOutput").ap()

    xqt_r = xqt.rearrange("(k p) s -> p k s", p=P)
    xkt_r = xkt.rearrange("(k p) s -> p k s", p=P)
    xvt_r = xvt.rearrange("(k p) s -> p k s", p=P)

    # at (A.T) buffering: double-buffer if it fits in SBUF alongside the rest
    at_bytes = SKT * SQ * 2
    fixed_bytes = (3 * 8 * E * 2        # weights
                   + 3 * 8 * 512 * 2    # x stream bufs
                   + 4 * SQ * 2 + 4 * SK * 2 + SKT * HL * 65 * 2  # qt/kt/v
                   + 4096)              # misc
    at_bufs = 2 if fixed_bytes + 2 * at_bytes < 190 * 1024 else 1

    with tile.TileContext(nc) as tc:
        with tc.tile_pool(name="const", bufs=1) as const, \
             tc.tile_pool(name="persist", bufs=1) as persist, \
             tc.tile_pool(name="xs", bufs=3) as xs, \
             tc.tile_pool(name="atp", bufs=at_bufs) as atp, \
             tc.tile_pool(name="small", bufs=6) as small, \
             tc.tile_pool(name="psq", bufs=1, space="PSUM") as psq, \
             tc.tile_pool(name="pso", bufs=2, space="PSUM") as pso, \
             tc.tile_pool(name="psp", bufs=2, space="PSUM") as psp:

            wq_sb = const.tile([P, KT, E], BF, tag="wq")
            wk_sb = const.tile([P, KT, E], BF, tag="wk")
            wv_sb = const.tile([P, KT, E], BF, tag="wv")
            maskq_sb = const.tile([P, SQT], F32, tag="mq")
            nc.sync.dma_start(out=wq_sb, in_=wq.rearrange("(k p) e -> p k e", p=P))
            nc.sync.dma_start(out=wk_sb, in_=wk.rearrange("(k p) e -> p k e", p=P))
            nc.sync.dma_start(out=wv_sb, in_=wv.rearrange("(k p) e -> p k e", p=P))
            nc.sync.dma_start(out=maskq_sb, in_=maskq.rearrange("t p -> p t"))

            qt_sb = persist.tile([P, 4, SQ], BF, tag="qt")
            kt_sb = persist.tile([P, 4, SK], BF, tag="kt")
            v_sb = persist.tile([P, SKT, HL, DH + 1], BF, tag="v")

            # ---- projections: qT, kT ([dh, seq], head-pair-major) ----
            for dst, x_r, w_sb, nchunks, stot in (
                (qt_sb, xqt_r, wq_sb, QC, SQ),
                (kt_sb, xkt_r, wk_sb, KC, SK),
            ):
                for c in range(nchunks):
                    c0 = 512 * c
                    ncols = min(512, stot - c0)
                    xt = xs.tile([P, KT, 512], BF, tag="x")
                    nc.sync.dma_start(out=xt[:, :, :ncols],
                                      in_=x_r[:, :, c0:c0 + ncols])
                    for p4 in range(4):
                        ps = psp.tile([P, 512], F32, tag="proj")
                        for k in range(KT):
                            nc.tensor.matmul(
                                ps[:, :ncols],
                                w_sb[:, k, P * p4:P * (p4 + 1)],
                                xt[:, k, :ncols],
                                start=(k == 0), stop=(k == KT - 1))
                        nc.vector.tensor_copy(out=dst[:, p4, c0:c0 + ncols],
                                              in_=ps[:, :ncols])

            # ---- projection: v ([seq, dh] natural) + mask column ----
            for m in range(SKT):
                xt = xs.tile([P, KT, 512], BF, tag="x")
                nc.sync.dma_start(out=xt[:, :, :P],
                                  in_=xvt_r[:, :, P * m:P * (m + 1)])
                ps = psp.tile([P, 512], F32, tag="proj")
                for k in range(KT):
                    nc.tensor.matmul(ps, xt[:, k, :P], wv_sb[:, k, :],
                                     start=(k == 0), stop=(k == KT - 1))
                nc.vector.tensor_copy(
                    out=v_sb[:, m, :, 0:DH],
                    in_=ps.rearrange("p (h d) -> p h d", h=HL))
                nc.sync.dma_start(out=v_sb[:, m, :, DH:DH + 1],
                                  in_=maskk8[P * m:P * (m + 1)])

            # ---- attention per head ----
            for h in range(HL):
                p4, half = h // 2, h % 2
                pb = DH * half
                at = atp.tile([P, SKT, SQ], BF, tag="at")
                for t in range(SKT):
                    ps = psq.tile([P, SQ], F32, tag="qk")
                    for c in range(QC):
                        c0 = 512 * c
                        ncols = min(512, SQ - c0)
                        nc.tensor.matmul(
                            ps[:, c0:c0 + ncols],
                            kt_sb[pb:pb + DH, p4, P * t:P * (t + 1)],
                            qt_sb[pb:pb + DH, p4, c0:c0 + ncols],
                            start=True, stop=True)
                    nc.scalar.activation(out=at[:, t, :], in_=ps, func=Exp,
                                         scale=0.125)
                for sq in range(SQT):
                    po = pso.tile([P, DH + 1], F32, tag="o")
                    for t in range(SKT):
                        nc.tensor.matmul(po,
                                         at[:, t, P * sq:P * (sq + 1)],
                                         v_sb[:, t, h, :],
                                         start=(t == 0), stop=(t == SKT - 1))
                    rc = small.tile([P, 1], F32, tag="rc")
                    sc = small.tile([P, 1], F32, tag="sc")
                    nc.vector.reciprocal(rc, po[:, DH:DH + 1])
                    nc.vector.tensor_mul(sc, rc, maskq_sb[:, sq:sq + 1])
                    ob = small.tile([P, DH], F32, tag="ob")
                    nc.vector.tensor_scalar_mul(ob, po[:, 0:DH], sc)
                    nc.sync.dma_start(
                        out=out[P * sq:P * (sq + 1), DH * h:DH * (h + 1)],
                        in_=ob)

    nc.compile()
    return nc


def _get_program(SQT, SKT):
    key = (SQT, SKT)
    if key not in _PROG_CACHE:
        _PROG_CACHE[key] = _build_program(SQT, SKT)
    return _PROG_CACHE[key]


def kernel(Q_seq, K_seq, V_seq, WQ, WK, WV, Q_len, V_len):
    global _last_nc, _last_in_maps
    _ensure_paths()
    from concourse.bass_utils import run_bass_kernel_spmd

    Q_seq = np.asarray(Q_seq, dtype=np.float32)
    K_seq = np.asarray(K_seq, dtype=np.float32)
    V_seq = np.asarray(V_seq, dtype=np.float32)
    WQ = np.asarray(WQ, dtype=np.float32)
    WK = np.asarray(WK, dtype=np.float32)
    WV = np.asarray(WV, dtype=np.float32)
    Q_len = np.asarray(Q_len).reshape(-1)
    V_len = np.asarray(V_len).reshape(-1)

    B, S, _ = Q_seq.shape
    BF = ml_dtypes.bfloat16

    SQT = max(1, math.ceil(int(Q_len.max()) / P))
    SKT = max(1, math.ceil(int(V_len.max()) / P))
    SQ, SK = SQT * P, SKT * P

    nc = _get_program(SQT, SKT)

    in_maps = []
    for c in range(NCORES):
        b, g = c // 2, c % 2
        ql, vl = int(Q_len[b]), int(V_len[b])
        mk = (np.arange(SK) < vl)
        xq = np.ascontiguousarray(Q_seq[b, :SQ].T).astype(BF)
        xk = np.ascontiguousarray(K_seq[b, :SK].T).astype(BF)
        xv = np.ascontiguousarray((V_seq[b, :SK] * mk[:, None]).T).astype(BF)
        in_maps.append({
            "xqt": xq,
            "xkt": xk,
            "xvt": xv,
            "wq": np.ascontiguousarray(WQ[:, E * g:E * (g + 1)]).astype(BF),
            "wk": np.ascontiguousarray(WK[:, E * g:E * (g + 1)]).astype(BF),
            "wv": np.ascontiguousarray(WV[:, E * g:E * (g + 1)]).astype(BF),
            "maskq": (np.arange(SQ) < ql).astype(np.float32).reshape(SQT, P),
            "maskk8": np.repeat(mk.astype(BF)[:, None], HL, axis=1)[..., None],
        })

    res = run_bass_kernel_spmd(nc, in_maps, core_ids=list(range(NCORES)))
    _last_nc, _last_in_maps = nc, in_maps

    full = np.zeros((B, S, 2 * E), dtype=np.float32)
    for c in range(NCORES):
        b, g = c // 2, c % 2
        o = res.results[c]["out"]
        # rows >= SQ are never written by the kernel; keep host zeros there
        full[b, :SQ, E * g:E * (g + 1)] = o[:SQ]
    return full
